# revision 6
# baseline (speedup 1.0000x reference)
"""GPT-2 style transformer block on 8 TRN2 NeuronCores.

Sharding: token-data-parallel. Each batch's 2048 tokens are split into 8
chunks of 256; core c owns batch c//4 and chunks {j, 7-j} (j = c%4) so
causal attention work is balanced. QKV/proj/MLP/LN are purely local; the
only collectives are four small AllGathers (k chunk0, v chunk0, k chunk1,
v chunk1) within each 4-core batch group, fp8 on the wire. Chunk-major
wire layout means kt<8 (everything q0 needs, and all of alpha below) is
available after the first two AGs. All bounce/unpack DMAs are p-major
contiguous (128 descriptors each).

Score matmuls are head-PAIR packed: heads 2p/2p+1 run as two concurrent
row-tiled K=64 matmuls (tile_position row groups 0/64, fp8 lhsT x bf16
rhs) writing the two banks of one [128,1024] PSUM tile; one exp covers
both heads, and the causal 0/1 mask is applied with a stride-0-broadcast
tensor_tensor so a single 512-wide mask slab serves both heads. proj is
pair-packed (K=128 = two heads' hd rows).

Attention is split alpha (kt<8) / beta (kt>=8): after alpha, q0's
(tokens 0:256) attention is final, so proj+LN2+MLP for q0 run while
beta's exp wall occupies the scalar engine. The v rides the wire with a
65th ones-column so softmax denominators come free in the av matmul;
softmax runs without max-subtraction; 1/sqrt(hd) is folded into w_q and
the v-bias into the proj bias.
"""

import os
import sys

sys.path.insert(0, "/opt/trn_rl_repo")

import numpy as np
import ml_dtypes

import concourse.bass as bass
import concourse.tile as tile
from concourse import bacc, mybir
from concourse.bass_utils import run_bass_kernel_spmd
from concourse.masks import make_identity

F32 = mybir.dt.float32
FP8 = mybir.dt.float8e4
BF16 = mybir.dt.bfloat16
BF = ml_dtypes.bfloat16

B, T, C, H, HD = 2, 2048, 768, 12, 64
EPS = 1e-5
NCORES = 8
CHUNK = 256            # global chunk size (tokens)
TLOC = 512             # local tokens per core (2 chunks)
NKT = T // 128         # 16 key tiles per batch
MASK_W = 8 * 512 + 8 * 256   # 6144

KH = 6 * 128 * 256     # k chunk: [128 p][6 ct][256 t]
VW = 780               # v wire row: 12 heads x 65 (64 + ones)
VH = 2 * 128 * VW      # v chunk: [128 p][2 tt][780]

GROUPS_A = [(kt,) for kt in range(8)]
GROUPS_B = [(8, 9), (10, 11), (12, 13), (14, 15)]

LAST_EXEC_NS = None
LAST_RESULTS = None
_CACHE = {}


def _build(add_qk_bias, add_proj_bias, add_fc2_bias):
    nc = bacc.Bacc("TRN2", target_bir_lowering=False, debug=False,
                   num_devices=NCORES)

    x_ext = nc.dram_tensor("x", [128, 4 * C], F32, kind="ExternalInput")
    wq_ext = nc.dram_tensor("wq", [128, 6 * C], BF16, kind="ExternalInput")
    wk_ext = nc.dram_tensor("wk", [128, 6 * C], BF16, kind="ExternalInput")
    wv_ext = nc.dram_tensor("wv", [128, 6 * C], BF16, kind="ExternalInput")
    wp_ext = nc.dram_tensor("wp", [128, 6 * C], BF16, kind="ExternalInput")
    wfc_ext = nc.dram_tensor("wfc", [128, 24 * 6 * 128], BF16,
                             kind="ExternalInput")
    wfc2_ext = nc.dram_tensor("wfc2", [128, 24 * C], BF16,
                              kind="ExternalInput")
    masks_ext = nc.dram_tensor("masks", [128, MASK_W], BF16,
                               kind="ExternalInput")
    bqk_ext = nc.dram_tensor("bqk", [2, C], F32, kind="ExternalInput")
    bfc_ext = nc.dram_tensor("bfc", [4 * C], F32, kind="ExternalInput")
    bout_ext = nc.dram_tensor("bout", [2, C], F32, kind="ExternalInput")
    out_ext = nc.dram_tensor("out", [TLOC, C], F32, kind="ExternalOutput")

    with tile.TileContext(nc) as tc:
        with tc.tile_pool(name="dram", bufs=1, space="DRAM") as dram, \
             tc.tile_pool(name="singles", bufs=1) as singles, \
             tc.tile_pool(name="persist", bufs=1) as persist, \
             tc.tile_pool(name="small", bufs=3) as small:

            kc_in = [dram.tile([KH], FP8, name=f"kc_in{i}")
                     for i in range(2)]
            kc_all = [dram.tile([4, KH], FP8, name=f"kc_all{i}")
                      for i in range(2)]
            vc_in = [dram.tile([VH], FP8, name=f"vc_in{i}")
                     for i in range(2)]
            vc_all = [dram.tile([4, VH], FP8, name=f"vc_all{i}")
                      for i in range(2)]

            # x first: LN1 needs it before anything else
            x_sb = persist.tile([128, 4, C], F32)     # local x, becomes xmid
            nc.sync.dma_start(out=x_sb, in_=x_ext.ap())

            ident = singles.tile([128, 128], BF16)
            make_identity(nc, ident)
            eps_sb = singles.tile([128, 1], F32)
            nc.vector.memset(eps_sb, EPS)
            ones_pad = singles.tile([128, 64], F32)
            nc.vector.memset(ones_pad, 0.0)
            nc.vector.memset(ones_pad[0:1, :], 1.0)
            d_sb = singles.tile([128, 2, TLOC], F32)
            nc.vector.memset(d_sb, 1.0)

            bqk_sb = singles.tile([128, 2, 6], F32)
            if add_qk_bias:
                nc.sync.dma_start(
                    out=bqk_sb,
                    in_=bqk_ext.ap().rearrange("b (m p) -> p b m", p=128))
            bout_sb = singles.tile([128, 2, C], F32)
            if add_proj_bias or add_fc2_bias:
                bc = bout_ext.ap()
                nc.sync.dma_start(
                    out=bout_sb,
                    in_=bass.AP(tensor=bc.tensor, offset=bc.offset,
                                ap=[[0, 128], bc.ap[0], bc.ap[1]]))

            masks_sb = persist.tile([128, MASK_W], BF16)
            hT = persist.tile([128, 6, TLOC], BF16)   # h^T, reused for h2^T
            qT = persist.tile([128, 6, TLOC], BF16)   # head-pair layout
            yT = persist.tile([128, 6, TLOC], BF16)   # head-pair layout
            wp_sb = persist.tile([128, 6, C], BF16)
            wfc2_sb = persist.tile([128, 24, C], BF16)
            bfc_sb = singles.tile([128, 24], F32)
            gT = persist.tile([128, 24, TLOC], FP8)

            def layernorm_to(pool, xt, dst, tagsuf):
                stats = pool.tile([128, 3, 6], F32, tag="st" + tagsuf,
                                  name="st" + tagsuf)
                for sg in range(3):
                    nc.vector.bn_stats(out=stats[:, sg, :],
                                       in_=xt[:, sg * 256:(sg + 1) * 256])
                mv = pool.tile([128, 2], F32, tag="mv" + tagsuf,
                               name="mv" + tagsuf)
                nc.vector.bn_aggr(out=mv, in_=stats)
                nc.scalar.activation(out=mv[:, 1:2], in_=mv[:, 1:2],
                                     func=mybir.ActivationFunctionType.Sqrt,
                                     bias=eps_sb)
                nc.vector.reciprocal(out=mv[:, 1:2], in_=mv[:, 1:2])
                nc.vector.tensor_scalar(out=dst, in0=xt,
                                        scalar1=mv[:, 0:1], scalar2=mv[:, 1:2],
                                        op0=mybir.AluOpType.subtract,
                                        op1=mybir.AluOpType.mult)

            # ---------------- LN1 + transpose + QKV + AGs ----------------
            with tc.tile_pool(name="ln", bufs=3) as lnp, \
                 tc.tile_pool(name="tp", bufs=2, space="PSUM") as tpp, \
                 tc.tile_pool(name="qkp", bufs=2, space="PSUM") as qkp, \
                 tc.tile_pool(name="vp", bufs=2, space="PSUM") as vpp, \
                 tc.tile_pool(name="vsb", bufs=1) as vsbp:

                kT = vsbp.tile([128, 2, 6, 256], FP8)   # chunk-major
                v_sb = vsbp.tile([128, 4, VW], FP8)     # tt-major, with ones
                v5 = v_sb[:].rearrange("p t (h e) -> p t h e", e=65)
                nc.vector.memset(v5[:, :, :, 64:65], 1.0)
                wk_sb = vsbp.tile([128, 6, C], BF16)
                wq_sb = vsbp.tile([128, 6, C], BF16)
                wv_sb = vsbp.tile([128, 6, C], BF16)
                for sb, ext in ((wk_sb, wk_ext), (wv_sb, wv_ext),
                                (wq_sb, wq_ext)):
                    nc.sync.dma_start(
                        out=sb, in_=ext.ap().rearrange("p (a c) -> p a c",
                                                       c=C))

                for t in range(4):
                    xn = lnp.tile([128, C], BF16, tag="xn")
                    layernorm_to(lnp, x_sb[:, t, :], xn, "1")
                    for ct in range(6):
                        pt = tpp.tile([128, 128], BF16, tag="tp")
                        nc.tensor.transpose(
                            pt, xn[:, ct * 128:(ct + 1) * 128], ident)
                        nc.scalar.copy(hT[:, ct, t * 128:(t + 1) * 128], pt)

                for m in range(6):
                    ps = qkp.tile([128, TLOC], F32, tag="qk", name="psk")
                    for k in range(6):
                        nc.tensor.matmul(
                            ps, lhsT=wk_sb[:, k, m * 128:(m + 1) * 128],
                            rhs=hT[:, k, :], start=(k == 0), stop=(k == 5))
                    for cch in range(2):
                        sl = slice(cch * 256, cch * 256 + 256)
                        if add_qk_bias:
                            nc.scalar.activation(
                                out=kT[:, cch, m, :], in_=ps[:, sl],
                                func=mybir.ActivationFunctionType.Copy,
                                bias=bqk_sb[:, 1, m:m + 1])
                        else:
                            nc.scalar.copy(kT[:, cch, m, :], ps[:, sl])

                # k chunk bounces + AG for chunk 0 right away
                for cch in range(2):
                    nc.sync.dma_start(
                        out=kc_in[cch][:].rearrange("(p ct t) -> p ct t",
                                                    p=128, t=256),
                        in_=kT[:, cch, :, :])
                nc.gpsimd.collective_compute(
                    "AllGather", mybir.AluOpType.bypass,
                    replica_groups=[[0, 1, 2, 3], [4, 5, 6, 7]],
                    ins=[kc_in[0][:].opt()], outs=[kc_all[0][:].opt()])

                for tt in range(4):
                    pv = vpp.tile([128, 1024], F32, tag="v")
                    for k in range(6):
                        nc.tensor.matmul(
                            pv[:, 0:384],
                            lhsT=hT[:, k, tt * 128:(tt + 1) * 128],
                            rhs=wv_sb[:, k, 0:384],
                            start=(k == 0), stop=(k == 5))
                        nc.tensor.matmul(
                            pv[:, 512:896],
                            lhsT=hT[:, k, tt * 128:(tt + 1) * 128],
                            rhs=wv_sb[:, k, 384:768],
                            start=(k == 0), stop=(k == 5))
                    nc.scalar.copy(
                        v5[:, tt, 0:6, 0:64],
                        pv[:, 0:384].rearrange("p (h e) -> p h e", e=64))
                    nc.scalar.copy(
                        v5[:, tt, 6:12, 0:64],
                        pv[:, 512:896].rearrange("p (h e) -> p h e", e=64))
                    if tt == 1 or tt == 3:
                        cch = tt // 2
                        nc.sync.dma_start(
                            out=vc_in[cch][:].rearrange(
                                "(p t d) -> p t d", p=128, d=VW),
                            in_=v_sb[:, 2 * cch:2 * cch + 2, :])
                        nc.gpsimd.collective_compute(
                            "AllGather", mybir.AluOpType.bypass,
                            replica_groups=[[0, 1, 2, 3], [4, 5, 6, 7]],
                            ins=[vc_in[cch][:].opt()],
                            outs=[vc_all[cch][:].opt()])
                        if cch == 0:
                            nc.gpsimd.collective_compute(
                                "AllGather", mybir.AluOpType.bypass,
                                replica_groups=[[0, 1, 2, 3], [4, 5, 6, 7]],
                                ins=[kc_in[1][:].opt()],
                                outs=[kc_all[1][:].opt()])

                # q (pair layout: m-tile = head pair)
                for m in range(6):
                    ps = qkp.tile([128, TLOC], F32, tag="qk")
                    for k in range(6):
                        nc.tensor.matmul(
                            ps, lhsT=wq_sb[:, k, m * 128:(m + 1) * 128],
                            rhs=hT[:, k, :], start=(k == 0), stop=(k == 5))
                    if add_qk_bias:
                        nc.scalar.activation(
                            out=qT[:, m, :], in_=ps,
                            func=mybir.ActivationFunctionType.Copy,
                            bias=bqk_sb[:, 0, m:m + 1])
                    else:
                        nc.scalar.copy(qT[:, m, :], ps)

                # deferred weight/mask DMAs (needed later than x/w{k,v,q})
                nc.sync.dma_start(out=masks_sb, in_=masks_ext.ap())
                nc.sync.dma_start(
                    out=wp_sb,
                    in_=wp_ext.ap().rearrange("p (a c) -> p a c", c=C))
                nc.sync.dma_start(
                    out=wfc2_sb,
                    in_=wfc2_ext.ap().rearrange("p (a c) -> p a c", c=C))
                nc.sync.dma_start(
                    out=bfc_sb,
                    in_=bfc_ext.ap().rearrange("(m p) -> p m", p=128))

            # ---------------- attention ----------------
            with tc.tile_pool(name="kch", bufs=1) as kchp, \
                 tc.tile_pool(name="vaug", bufs=1) as vaugp, \
                 tc.tile_pool(name="esb", bufs=4) as esbp, \
                 tc.tile_pool(name="p1", bufs=1) as p1p:

                # k_ch: [p][chunk 2][rank 4][ct 6][256]
                k_ch = kchp.tile([128, 2, 4, 6, 256], FP8)
                # va4: [p][chunk 2][rank 4][tt 2][12*65] as wired
                v_ch = vaugp.tile([128, 2, 4, 2, VW], FP8)
                part1 = p1p.tile([65, 12, 256], F32)

                for cch in range(2):
                    for r in range(4):
                        nc.sync.dma_start(
                            out=k_ch[:, cch, r, :, :],
                            in_=kc_all[cch][r].rearrange(
                                "(p ct t) -> p ct t", p=128, t=256))
                        nc.gpsimd.dma_start(
                            out=v_ch[:, cch, r, :, :],
                            in_=vc_all[cch][r].rearrange(
                                "(p t d) -> p t d", p=128, d=VW))

                def k_lhsT(kt, p, half):
                    cch = 0 if kt < 8 else 1
                    ck = kt // 2
                    r = ck if ck < 4 else 7 - ck
                    loc = (kt % 2) * 128
                    return k_ch[64 * half:64 * half + 64, cch, r, p,
                                loc:loc + 128]

                def v_lhsT(kt, h):
                    cch = 0 if kt < 8 else 1
                    ck = kt // 2
                    r = ck if ck < 4 else 7 - ck
                    t = kt % 2
                    return v_ch[:, cch, r, t, h * 65:(h + 1) * 65]

                def finalize(h, src, q0, qn, pool):
                    h2, p = h % 2, h // 2
                    nc.vector.tensor_copy(d_sb[0:1, h2, q0:q0 + qn],
                                          src[64:65, :])
                    pb = pool.tile([64, qn], F32, tag="e", name="pbc")
                    nc.tensor.matmul(pb, lhsT=ones_pad,
                                     rhs=d_sb[:, h2, q0:q0 + qn],
                                     start=True, stop=True)
                    b_sb = small.tile([64, 256], F32, tag="bsb", name="bsb")
                    nc.vector.reciprocal_approx_fast(out=b_sb[:, 0:qn],
                                                     in_=pb)
                    nc.vector.tensor_mul(yT[64 * h2:64 * h2 + 64, p,
                                            q0:q0 + qn],
                                         src[0:64, :], b_sb[:, 0:qn])

                def do_group(p, g, pool, epool, q_sl, pav_w):
                    pe = epool.tile([128, 1024], F32, tag="e")
                    if len(g) == 1:
                        kt = g[0]
                        for half in range(2):
                            nc.tensor.matmul(
                                pe[:, half * 512:half * 512 + 512],
                                lhsT=k_lhsT(kt, p, half),
                                rhs=qT[64 * half:64 * half + 64, p, :],
                                start=True, stop=True)
                        moff = kt * 512
                    else:
                        for half in range(2):
                            for i, kt in enumerate(g):
                                so = half * 512 + i * 256
                                nc.tensor.matmul(
                                    pe[:, so:so + 256],
                                    lhsT=k_lhsT(kt, p, half),
                                    rhs=qT[64 * half:64 * half + 64, p,
                                           256:512],
                                    start=True, stop=True)
                        moff = 4096 + (g[0] - 8) * 256
                    e_sb = esbp.tile([128, 1024], BF16, tag="esb")
                    nc.scalar.activation(
                        out=e_sb, in_=pe,
                        func=mybir.ActivationFunctionType.Exp)
                    ms = masks_sb[:, moff:moff + 512]
                    mb = bass.AP(tensor=ms.tensor, offset=ms.offset,
                                 ap=[ms.ap[0], [0, 2], ms.ap[1]])
                    e3 = e_sb[:].rearrange("p (a c) -> p a c", a=2)
                    nc.vector.tensor_mul(e3, e3, mb)
                    return e_sb

                # ---- alpha: kt<8 (everything q0 needs) ----
                with tc.tile_pool(name="epA", bufs=2, space="PSUM") as eppA, \
                     tc.tile_pool(name="avA", bufs=4, space="PSUM") as avpA:

                    pavs = {}
                    pends = []

                    def emit_av_a(pend):
                        p, e_sb, g = pend
                        kt = g[0]
                        for h2 in range(2):
                            h = 2 * p + h2
                            nc.tensor.matmul(
                                pavs[h], lhsT=v_lhsT(kt, h),
                                rhs=e_sb[:, h2 * 512:h2 * 512 + 512],
                                start=(kt == 0), stop=(kt == 7),
                                skip_group_check=True)
                        if kt == 7:
                            for h2 in range(2):
                                h = 2 * p + h2
                                finalize(h, pavs[h][:, 0:256], 0, 256, eppA)
                                nc.vector.tensor_copy(part1[:, h, :],
                                                      pavs[h][:, 256:512])
                                del pavs[h]

                    for p in range(6):
                        for h2 in range(2):
                            pavs[2 * p + h2] = avpA.tile(
                                [65, TLOC], F32, tag="av",
                                name=f"pav{2 * p + h2}")
                        for g in GROUPS_A:
                            e_sb = do_group(p, g, avpA, eppA, None, None)
                            pends.append((p, e_sb, g))
                            if len(pends) > 3:
                                emit_av_a(pends.pop(0))
                    for pend in pends:
                        emit_av_a(pend)
                    pends = []

                # ---- proj + residual + LN2 for q0 (t = 0,1) ----
                def proj_ln2(trange):
                    with tc.tile_pool(name="pp", bufs=2,
                                      space="PSUM") as ppp, \
                         tc.tile_pool(name="ln2", bufs=2) as ln2p, \
                         tc.tile_pool(name="tp2", bufs=2,
                                      space="PSUM") as tpp2:
                        for t in trange:
                            pp = ppp.tile([128, C], F32, tag="pp")
                            for p in range(6):
                                y_ap = yT[:, p, t * 128:(t + 1) * 128]
                                nc.tensor.matmul(pp[:, 0:512], lhsT=y_ap,
                                                 rhs=wp_sb[:, p, 0:512],
                                                 start=(p == 0),
                                                 stop=(p == 5))
                                nc.tensor.matmul(pp[:, 512:768], lhsT=y_ap,
                                                 rhs=wp_sb[:, p, 512:768],
                                                 start=(p == 0),
                                                 stop=(p == 5))
                            nc.vector.tensor_add(x_sb[:, t, :],
                                                 x_sb[:, t, :], pp)
                            if add_proj_bias:
                                nc.vector.tensor_add(x_sb[:, t, :],
                                                     x_sb[:, t, :],
                                                     bout_sb[:, 0, :])
                            xn2 = ln2p.tile([128, C], BF16, tag="xn2",
                                            name="xn2")
                            layernorm_to(ln2p, x_sb[:, t, :], xn2, "2")
                            for ct in range(6):
                                pt = tpp2.tile([128, 128], BF16, tag="tp2")
                                nc.tensor.transpose(
                                    pt, xn2[:, ct * 128:(ct + 1) * 128],
                                    ident)
                                nc.vector.tensor_copy(
                                    hT[:, ct, t * 128:(t + 1) * 128], pt)

                proj_ln2((0, 1))

                # ---- beta: kt>=8 (q1's second half of keys) ----
                with tc.tile_pool(name="epB", bufs=2, space="PSUM") as eppB, \
                     tc.tile_pool(name="avB", bufs=2, space="PSUM") as avpB:

                    pav2 = {}
                    pends = []

                    def emit_av_b(pend):
                        p, e_sb, g = pend
                        for h2 in range(2):
                            h = 2 * p + h2
                            for i, kt in enumerate(g):
                                so = h2 * 512 + i * 256
                                nc.tensor.matmul(
                                    pav2[h], lhsT=v_lhsT(kt, h),
                                    rhs=e_sb[:, so:so + 256],
                                    start=(kt == 8), stop=(kt == 15),
                                    skip_group_check=True)
                        if g[-1] == 15:
                            for h2 in range(2):
                                h = 2 * p + h2
                                sum_sb = small.tile([65, 256], F32,
                                                    tag="sum", name="sum")
                                nc.vector.tensor_add(sum_sb, pav2[h],
                                                     part1[:, h, :])
                                finalize(h, sum_sb, 256, 256, eppB)
                                del pav2[h]

                    for p in range(6):
                        for h2 in range(2):
                            pav2[2 * p + h2] = avpB.tile(
                                [65, 256], F32, tag="av2",
                                name=f"pav2_{2 * p + h2}")
                        for g in GROUPS_B:
                            e_sb = do_group(p, g, avpB, eppB, None, None)
                            pends.append((p, e_sb, g))
                            if len(pends) > 3:
                                emit_av_b(pends.pop(0))
                    for pend in pends:
                        emit_av_b(pend)
                    pends = []

                # ---- MLP for q0 (overlaps beta's scalar-engine wall) ----
                def mlp(trange):
                    t0 = trange[0] * 128
                    tn = len(trange) * 128
                    with tc.tile_pool(name="wfc", bufs=6) as wfcp, \
                         tc.tile_pool(name="fcp", bufs=2,
                                      space="PSUM") as fcpp:
                        wfc_t = wfc_ext.ap().rearrange(
                            "p (m k j) -> p m (k j)", m=24, k=6)
                        for m in range(24):
                            wt = wfcp.tile([128, 6, 128], BF16, tag="wfc")
                            nc.sync.dma_start(out=wt, in_=wfc_t[:, m, :])
                            pf = fcpp.tile([128, 256], F32, tag="fc")
                            for k in range(6):
                                nc.tensor.matmul(pf, lhsT=wt[:, k, :],
                                                 rhs=hT[:, k, t0:t0 + tn],
                                                 start=(k == 0),
                                                 stop=(k == 5))
                            nc.scalar.activation(
                                out=gT[:, m, t0:t0 + tn], in_=pf,
                                func=mybir.ActivationFunctionType
                                .Gelu_apprx_tanh,
                                bias=bfc_sb[:, m:m + 1])
                    with tc.tile_pool(name="f2p", bufs=1,
                                      space="PSUM") as f2pp, \
                         tc.tile_pool(name="osb", bufs=2) as osbp:
                        pf2s = {t: f2pp.tile([128, C], F32, tag=f"f2_{t}",
                                             name=f"pf2_{t}")
                                for t in trange}
                        for k in range(24):
                            for t in trange:
                                nc.tensor.matmul(
                                    pf2s[t][:, 0:512],
                                    lhsT=gT[:, k, t * 128:(t + 1) * 128],
                                    rhs=wfc2_sb[:, k, 0:512],
                                    start=(k == 0), stop=(k == 23))
                                nc.tensor.matmul(
                                    pf2s[t][:, 512:768],
                                    lhsT=gT[:, k, t * 128:(t + 1) * 128],
                                    rhs=wfc2_sb[:, k, 512:768],
                                    start=(k == 0), stop=(k == 23))
                        for t in trange:
                            o_sb = osbp.tile([128, C], F32, tag="osb",
                                             name="osb")
                            nc.vector.tensor_add(o_sb, x_sb[:, t, :],
                                                 pf2s[t])
                            if add_fc2_bias:
                                nc.vector.tensor_add(o_sb, o_sb,
                                                     bout_sb[:, 1, :])
                            nc.sync.dma_start(
                                out=out_ext[t * 128:(t + 1) * 128, :],
                                in_=o_sb)

                mlp((0, 1))
                proj_ln2((2, 3))
                mlp((2, 3))

    nc.compile()
    return nc


def _preprocess(inputs):
    f = lambda k: np.asarray(inputs[k], np.float32)
    x = f("x"); w_attn = f("w_attn"); b_attn = f("b_attn")
    w_proj = f("w_proj"); b_proj = f("b_proj")
    w_fc = f("w_fc"); b_fc = f("b_fc"); w_fc2 = f("w_fc2"); b_fc2 = f("b_fc2")
    ln1_g = f("ln1_g"); ln1_b = f("ln1_b"); ln2_g = f("ln2_g"); ln2_b = f("ln2_b")

    w_attn_eff = ln1_g[:, None] * w_attn
    b_attn_eff = b_attn + ln1_b @ w_attn
    s = 1.0 / np.sqrt(HD)
    w_q = w_attn_eff[:, 0:C] * s
    w_k = w_attn_eff[:, C:2 * C]
    w_v = w_attn_eff[:, 2 * C:3 * C]
    b_q = b_attn_eff[0:C] * s
    b_k = b_attn_eff[C:2 * C]
    b_v = b_attn_eff[2 * C:3 * C]
    b_proj_eff = b_proj + b_v @ w_proj
    w_fc_eff = ln2_g[:, None] * w_fc
    b_fc_eff = b_fc + ln2_b @ w_fc

    def pmajor(w, blocks):   # [C_in, N] -> [128, blocks * N/blocks...]
        # w: [(blocks*128), N] -> [128, blocks, N] p-major
        n = w.shape[1]
        return np.ascontiguousarray(
            w.reshape(blocks, 128, n).transpose(1, 0, 2).reshape(
                128, blocks * n).astype(BF))

    wq16 = pmajor(w_q, 6)
    wk16 = pmajor(w_k, 6)
    wv16 = pmajor(w_v, 6)
    wp16 = pmajor(w_proj, 6)
    # wfc: [128, 24 m, 6 k, 128] p-major
    wfc16 = np.ascontiguousarray(
        w_fc_eff.reshape(6, 128, 24, 128).transpose(1, 2, 0, 3).reshape(
            128, 24 * 6 * 128).astype(BF))
    wfc216 = pmajor(w_fc2, 24)

    bqk = np.stack([b_q, b_k]).astype(np.float32)
    bout = np.stack([b_proj_eff, b_fc2]).astype(np.float32)

    flags = (bool(np.any(bqk != 0)), bool(np.any(b_proj_eff != 0)),
             bool(np.any(b_fc2 != 0)))

    # mask slab [128, 6144] per core-group position j
    kpos = np.arange(128)
    qpos = np.arange(CHUNK)
    masks = np.zeros((4, 128, MASK_W), np.float32)
    for j in range(4):
        for kt in range(NKT):
            gk = kt * 128 + kpos[:, None]
            if kt < 8:
                off = kt * 512
                gq0 = j * CHUNK + qpos[None, :]
                gq1 = (7 - j) * CHUNK + qpos[None, :]
                masks[j, :, off:off + 256] = (gq0 >= gk)
                masks[j, :, off + 256:off + 512] = (gq1 >= gk)
            else:
                off = 4096 + (kt - 8) * 256
                gq1 = (7 - j) * CHUNK + qpos[None, :]
                masks[j, :, off:off + 256] = (gq1 >= gk)
    masks16 = masks.astype(BF)

    in_maps = []
    for c in range(NCORES):
        b, j = c // 4, c % 4
        x_loc = np.concatenate(
            [x[b, j * CHUNK:(j + 1) * CHUNK],
             x[b, (7 - j) * CHUNK:(8 - j) * CHUNK]]).astype(np.float32)
        x_pm = np.ascontiguousarray(
            x_loc.reshape(4, 128, C).transpose(1, 0, 2).reshape(128, 4 * C))
        in_maps.append({
            "x": x_pm,
            "wq": wq16, "wk": wk16, "wv": wv16, "wp": wp16,
            "wfc": wfc16, "wfc2": wfc216,
            "masks": np.ascontiguousarray(masks16[j]),
            "bqk": bqk, "bfc": b_fc_eff.astype(np.float32), "bout": bout,
        })
    return in_maps, flags


def kernel(**inputs):
    global LAST_EXEC_NS, LAST_RESULTS
    in_maps, flags = _preprocess(inputs)
    if flags not in _CACHE:
        _CACHE[flags] = _build(*flags)
    nc = _CACHE[flags]
    trace = bool(os.environ.get("BASS_KERNEL_TRACE"))
    res = run_bass_kernel_spmd(nc, in_maps, core_ids=list(range(NCORES)),
                               trace=trace)
    LAST_EXEC_NS = res.exec_time_ns
    LAST_RESULTS = res
    out = np.empty((B, T, C), np.float32)
    for c in range(NCORES):
        b, j = c // 4, c % 4
        o = res.results[c]["out"]
        out[b, j * CHUNK:(j + 1) * CHUNK] = o[0:CHUNK]
        out[b, (7 - j) * CHUNK:(8 - j) * CHUNK] = o[CHUNK:TLOC]
    return out


# revision 10
# speedup vs baseline: 1.0831x; 1.0831x over previous
"""GPT-2 style transformer block on 8 TRN2 NeuronCores.

Sharding: token-data-parallel. Each batch's 2048 tokens are split into 8
chunks of 256; core c owns batch c//4 and chunks {j, 7-j} (j = c%4) so
causal attention work is balanced. QKV/proj/MLP/LN are purely local; the
only collectives are four small AllGathers (k chunk0, v chunk0, k chunk1,
v chunk1) within each 4-core batch group, fp8 on the wire. Chunk-major
wire layout means kt<8 (everything q0 needs, and all of alpha below) is
available after the first two AGs. All bounce/unpack DMAs are p-major
contiguous (128 descriptors each).

Score matmuls are head-PAIR packed: heads 2p/2p+1 run as two concurrent
row-tiled K=64 matmuls (tile_position row groups 0/64, fp8 lhsT x bf16
rhs) writing the two banks of one [128,1024] PSUM tile; one exp covers
both heads, and the causal 0/1 mask is applied with a stride-0-broadcast
tensor_tensor so a single 512-wide mask slab serves both heads. proj is
pair-packed (K=128 = two heads' hd rows).

Attention is split alpha (kt<8) / beta (kt>=8): after alpha, q0's
(tokens 0:256) attention is final, so proj+LN2+MLP for q0 run while
beta's exp wall occupies the scalar engine. The v rides the wire with a
65th ones-column so softmax denominators come free in the av matmul;
softmax runs without max-subtraction; 1/sqrt(hd) is folded into w_q and
the v-bias into the proj bias.
"""

import os
import sys

sys.path.insert(0, "/opt/trn_rl_repo")

import numpy as np
import ml_dtypes

import concourse.bass as bass
import concourse.tile as tile
from concourse import bacc, mybir
from concourse.bass_utils import run_bass_kernel_spmd
from concourse.masks import make_identity

F32 = mybir.dt.float32
FP8 = mybir.dt.float8e4
BF16 = mybir.dt.bfloat16
BF = ml_dtypes.bfloat16

B, T, C, H, HD = 2, 2048, 768, 12, 64
EPS = 1e-5
NCORES = 8
CHUNK = 256            # global chunk size (tokens)
TLOC = 512             # local tokens per core (2 chunks)
NKT = T // 128         # 16 key tiles per batch
MASK_W = 8 * 512 + 8 * 256   # 6144

KH = 6 * 128 * 256     # k chunk: [128 p][6 ct][256 t]
VW = 780               # v wire row: 12 heads x 65 (64 + ones)
VH = 2 * 128 * VW      # v chunk: [128 p][2 tt][780]

GROUPS_A = [(kt,) for kt in range(8)]
GROUPS_B = [(8, 9), (10, 11), (12, 13), (14, 15)]

LAST_EXEC_NS = None
LAST_RESULTS = None
_CACHE = {}


def _build(add_qk_bias, add_proj_bias, add_fc2_bias):
    nc = bacc.Bacc("TRN2", target_bir_lowering=False, debug=False,
                   num_devices=NCORES)

    x_ext = nc.dram_tensor("x", [128, 4 * C], F32, kind="ExternalInput")
    wq_ext = nc.dram_tensor("wq", [128, 6 * C], BF16, kind="ExternalInput")
    wk_ext = nc.dram_tensor("wk", [128, 6 * C], BF16, kind="ExternalInput")
    wv_ext = nc.dram_tensor("wv", [128, 6 * C], BF16, kind="ExternalInput")
    wp_ext = nc.dram_tensor("wp", [128, 6 * C], BF16, kind="ExternalInput")
    wfc_ext = nc.dram_tensor("wfc", [128, 24 * 6 * 128], BF16,
                             kind="ExternalInput")
    wfc2_ext = nc.dram_tensor("wfc2", [128, 24 * C], BF16,
                              kind="ExternalInput")
    masks_ext = nc.dram_tensor("masks", [128, MASK_W], BF16,
                               kind="ExternalInput")
    bqk_ext = nc.dram_tensor("bqk", [2, C], F32, kind="ExternalInput")
    bfc_ext = nc.dram_tensor("bfc", [4 * C], F32, kind="ExternalInput")
    bout_ext = nc.dram_tensor("bout", [2, C], F32, kind="ExternalInput")
    out_ext = nc.dram_tensor("out", [TLOC, C], F32, kind="ExternalOutput")

    with tile.TileContext(nc) as tc:
        with tc.tile_pool(name="dram", bufs=1, space="DRAM") as dram, \
             tc.tile_pool(name="singles", bufs=1) as singles, \
             tc.tile_pool(name="persist", bufs=1) as persist, \
             tc.tile_pool(name="small", bufs=3) as small:

            kc_in = [dram.tile([KH], FP8, name=f"kc_in{i}")
                     for i in range(2)]
            kc_all = [dram.tile([4, KH], FP8, name=f"kc_all{i}")
                      for i in range(2)]
            vc_in = [dram.tile([VH], FP8, name=f"vc_in{i}")
                     for i in range(2)]
            vc_all = [dram.tile([4, VH], FP8, name=f"vc_all{i}")
                      for i in range(2)]

            # x first: LN1 needs it before anything else
            x_sb = persist.tile([128, 4, C], F32)     # local x, becomes xmid
            for t in range(4):
                nc.sync.dma_start(out=x_sb[:, t, :],
                                  in_=x_ext[:, t * C:(t + 1) * C])

            ident = singles.tile([128, 128], BF16)
            make_identity(nc, ident)
            eps_sb = singles.tile([128, 1], F32)
            nc.vector.memset(eps_sb, EPS)
            ones_pad = singles.tile([128, 64], F32)
            nc.vector.memset(ones_pad, 0.0)
            nc.vector.memset(ones_pad[0:1, :], 1.0)
            d_sb = singles.tile([128, 2, TLOC], F32)
            nc.vector.memset(d_sb, 1.0)

            bqk_sb = singles.tile([128, 2, 6], F32)
            if add_qk_bias:
                nc.sync.dma_start(
                    out=bqk_sb,
                    in_=bqk_ext.ap().rearrange("b (m p) -> p b m", p=128))
            bout_sb = singles.tile([128, 2, C], F32)
            if add_proj_bias or add_fc2_bias:
                bc = bout_ext.ap()
                nc.sync.dma_start(
                    out=bout_sb,
                    in_=bass.AP(tensor=bc.tensor, offset=bc.offset,
                                ap=[[0, 128], bc.ap[0], bc.ap[1]]))

            masks_sb = persist.tile([128, MASK_W], BF16)
            hT = persist.tile([128, 6, TLOC], BF16)   # h^T, reused for h2^T
            qT = persist.tile([128, 6, TLOC], BF16)   # head-pair layout
            yT = persist.tile([128, 6, TLOC], BF16)   # head-pair layout
            wp_sb = persist.tile([128, 6, C], BF16)
            wfc2_sb = persist.tile([128, 24, C], BF16)
            bfc_sb = singles.tile([128, 24], F32)
            gT = persist.tile([128, 24, TLOC], BF16)

            def layernorm_to(pool, xt, dst, tagsuf):
                stats = pool.tile([128, 3, 6], F32, tag="st" + tagsuf,
                                  name="st" + tagsuf)
                for sg in range(3):
                    nc.vector.bn_stats(out=stats[:, sg, :],
                                       in_=xt[:, sg * 256:(sg + 1) * 256])
                mv = pool.tile([128, 2], F32, tag="mv" + tagsuf,
                               name="mv" + tagsuf)
                nc.vector.bn_aggr(out=mv, in_=stats)
                nc.scalar.activation(out=mv[:, 1:2], in_=mv[:, 1:2],
                                     func=mybir.ActivationFunctionType.Sqrt,
                                     bias=eps_sb)
                nc.vector.reciprocal(out=mv[:, 1:2], in_=mv[:, 1:2])
                nc.vector.tensor_scalar(out=dst, in0=xt,
                                        scalar1=mv[:, 0:1], scalar2=mv[:, 1:2],
                                        op0=mybir.AluOpType.subtract,
                                        op1=mybir.AluOpType.mult)

            # ---------------- LN1 + transpose + QKV + AGs ----------------
            with tc.tile_pool(name="ln", bufs=3) as lnp, \
                 tc.tile_pool(name="tp", bufs=2, space="PSUM") as tpp, \
                 tc.tile_pool(name="qkp", bufs=2, space="PSUM") as qkp, \
                 tc.tile_pool(name="vp", bufs=2, space="PSUM") as vpp, \
                 tc.tile_pool(name="vsb", bufs=1) as vsbp:

                kT = vsbp.tile([128, 2, 6, 256], FP8)   # chunk-major
                v_sb = vsbp.tile([128, 4, VW], FP8)     # tt-major, with ones
                v5 = v_sb[:].rearrange("p t (h e) -> p t h e", e=65)
                nc.vector.memset(v5[:, :, :, 64:65], 1.0)
                wk_sb = vsbp.tile([128, 6, C], BF16)
                wq_sb = vsbp.tile([128, 6, C], BF16)
                wv_sb = vsbp.tile([128, 6, C], BF16)
                for sb, ext in ((wk_sb, wk_ext), (wv_sb, wv_ext),
                                (wq_sb, wq_ext)):
                    nc.sync.dma_start(
                        out=sb, in_=ext.ap().rearrange("p (a c) -> p a c",
                                                       c=C))

                for t in range(4):
                    xn = lnp.tile([128, C], BF16, tag="xn")
                    layernorm_to(lnp, x_sb[:, t, :], xn, "1")
                    for ct in range(6):
                        pt = tpp.tile([128, 128], BF16, tag="tp")
                        nc.tensor.transpose(
                            pt, xn[:, ct * 128:(ct + 1) * 128], ident)
                        nc.scalar.copy(hT[:, ct, t * 128:(t + 1) * 128], pt)

                def k_chunk(cch):
                    sl = slice(cch * 256, cch * 256 + 256)
                    for m in range(6):
                        ps = qkp.tile([128, 256], F32, tag="qk", name="psk")
                        for k in range(6):
                            nc.tensor.matmul(
                                ps, lhsT=wk_sb[:, k, m * 128:(m + 1) * 128],
                                rhs=hT[:, k, sl], start=(k == 0),
                                stop=(k == 5))
                        if add_qk_bias:
                            nc.scalar.activation(
                                out=kT[:, cch, m, :], in_=ps,
                                func=mybir.ActivationFunctionType.Copy,
                                bias=bqk_sb[:, 1, m:m + 1])
                        else:
                            nc.scalar.copy(kT[:, cch, m, :], ps)
                    nc.sync.dma_start(
                        out=kc_in[cch][:].rearrange("(p ct t) -> p ct t",
                                                    p=128, t=256),
                        in_=kT[:, cch, :, :])
                    nc.gpsimd.collective_compute(
                        "AllGather", mybir.AluOpType.bypass,
                        replica_groups=[[0, 1, 2, 3], [4, 5, 6, 7]],
                        ins=[kc_in[cch][:].opt()],
                        outs=[kc_all[cch][:].opt()])

                def v_chunk(cch):
                    for tt in (2 * cch, 2 * cch + 1):
                        pv = vpp.tile([128, 1024], F32, tag="v")
                        for k in range(6):
                            nc.tensor.matmul(
                                pv[:, 0:384],
                                lhsT=hT[:, k, tt * 128:(tt + 1) * 128],
                                rhs=wv_sb[:, k, 0:384],
                                start=(k == 0), stop=(k == 5))
                            nc.tensor.matmul(
                                pv[:, 512:896],
                                lhsT=hT[:, k, tt * 128:(tt + 1) * 128],
                                rhs=wv_sb[:, k, 384:768],
                                start=(k == 0), stop=(k == 5))
                        nc.scalar.copy(
                            v5[:, tt, 0:6, 0:64],
                            pv[:, 0:384].rearrange("p (h e) -> p h e", e=64))
                        nc.scalar.copy(
                            v5[:, tt, 6:12, 0:64],
                            pv[:, 512:896].rearrange("p (h e) -> p h e",
                                                     e=64))
                    nc.sync.dma_start(
                        out=vc_in[cch][:].rearrange(
                            "(p t d) -> p t d", p=128, d=VW),
                        in_=v_sb[:, 2 * cch:2 * cch + 2, :])
                    nc.gpsimd.collective_compute(
                        "AllGather", mybir.AluOpType.bypass,
                        replica_groups=[[0, 1, 2, 3], [4, 5, 6, 7]],
                        ins=[vc_in[cch][:].opt()],
                        outs=[vc_all[cch][:].opt()])

                k_chunk(0)
                v_chunk(0)
                k_chunk(1)
                v_chunk(1)

                # q (pair layout: m-tile = head pair)
                for m in range(6):
                    ps = qkp.tile([128, TLOC], F32, tag="qk")
                    for k in range(6):
                        nc.tensor.matmul(
                            ps, lhsT=wq_sb[:, k, m * 128:(m + 1) * 128],
                            rhs=hT[:, k, :], start=(k == 0), stop=(k == 5))
                    if add_qk_bias:
                        nc.scalar.activation(
                            out=qT[:, m, :], in_=ps,
                            func=mybir.ActivationFunctionType.Copy,
                            bias=bqk_sb[:, 0, m:m + 1])
                    else:
                        nc.scalar.copy(qT[:, m, :], ps)

                # deferred weight/mask DMAs (needed later than x/w{k,v,q})
                nc.sync.dma_start(out=masks_sb, in_=masks_ext.ap())
                nc.sync.dma_start(
                    out=wp_sb,
                    in_=wp_ext.ap().rearrange("p (a c) -> p a c", c=C))
                nc.sync.dma_start(
                    out=wfc2_sb,
                    in_=wfc2_ext.ap().rearrange("p (a c) -> p a c", c=C))
                nc.sync.dma_start(
                    out=bfc_sb,
                    in_=bfc_ext.ap().rearrange("(m p) -> p m", p=128))

            # ---------------- attention ----------------
            with tc.tile_pool(name="kch", bufs=1) as kchp, \
                 tc.tile_pool(name="vaug", bufs=1) as vaugp, \
                 tc.tile_pool(name="esb", bufs=4) as esbp, \
                 tc.tile_pool(name="p1", bufs=1) as p1p:

                # k_ch: [p][chunk 2][rank 4][ct 6][256]
                k_ch = kchp.tile([128, 2, 4, 6, 256], FP8)
                # va4: [p][chunk 2][rank 4][tt 2][12*65] as wired
                v_ch = vaugp.tile([128, 2, 4, 2, VW], FP8)
                part1 = p1p.tile([65, 12, 256], F32)

                for cch in range(2):
                    for r in range(4):
                        nc.sync.dma_start(
                            out=k_ch[:, cch, r, :, :],
                            in_=kc_all[cch][r].rearrange(
                                "(p ct t) -> p ct t", p=128, t=256))
                        nc.gpsimd.dma_start(
                            out=v_ch[:, cch, r, :, :],
                            in_=vc_all[cch][r].rearrange(
                                "(p t d) -> p t d", p=128, d=VW))

                def k_lhsT(kt, p, half):
                    cch = 0 if kt < 8 else 1
                    ck = kt // 2
                    r = ck if ck < 4 else 7 - ck
                    loc = (kt % 2) * 128
                    return k_ch[64 * half:64 * half + 64, cch, r, p,
                                loc:loc + 128]

                def v_lhsT(kt, h):
                    cch = 0 if kt < 8 else 1
                    ck = kt // 2
                    r = ck if ck < 4 else 7 - ck
                    t = kt % 2
                    return v_ch[:, cch, r, t, h * 65:(h + 1) * 65]

                def finalize(h, src, q0, qn, pool):
                    h2, p = h % 2, h // 2
                    nc.vector.tensor_copy(d_sb[0:1, h2, q0:q0 + qn],
                                          src[64:65, :])
                    pb = pool.tile([64, qn], F32, tag="e", name="pbc")
                    nc.tensor.matmul(pb, lhsT=ones_pad,
                                     rhs=d_sb[:, h2, q0:q0 + qn],
                                     start=True, stop=True)
                    b_sb = small.tile([64, 256], F32, tag="bsb", name="bsb")
                    nc.vector.reciprocal_approx_fast(out=b_sb[:, 0:qn],
                                                     in_=pb)
                    nc.vector.tensor_mul(yT[64 * h2:64 * h2 + 64, p,
                                            q0:q0 + qn],
                                         src[0:64, :], b_sb[:, 0:qn])

                def do_group(p, g, pool, epool, q_sl, pav_w):
                    pe = epool.tile([128, 1024], F32, tag="e")
                    if len(g) == 1:
                        kt = g[0]
                        for half in range(2):
                            nc.tensor.matmul(
                                pe[:, half * 512:half * 512 + 512],
                                lhsT=k_lhsT(kt, p, half),
                                rhs=qT[64 * half:64 * half + 64, p, :],
                                start=True, stop=True)
                        moff = kt * 512
                    else:
                        for half in range(2):
                            for i, kt in enumerate(g):
                                so = half * 512 + i * 256
                                nc.tensor.matmul(
                                    pe[:, so:so + 256],
                                    lhsT=k_lhsT(kt, p, half),
                                    rhs=qT[64 * half:64 * half + 64, p,
                                           256:512],
                                    start=True, stop=True)
                        moff = 4096 + (g[0] - 8) * 256
                    e_sb = esbp.tile([128, 1024], BF16, tag="esb")
                    nc.scalar.activation(
                        out=e_sb, in_=pe,
                        func=mybir.ActivationFunctionType.Exp)
                    ms = masks_sb[:, moff:moff + 512]
                    mb = bass.AP(tensor=ms.tensor, offset=ms.offset,
                                 ap=[ms.ap[0], [0, 2], ms.ap[1]])
                    e3 = e_sb[:].rearrange("p (a c) -> p a c", a=2)
                    nc.vector.tensor_mul(e3, e3, mb)
                    return e_sb

                def mlp_fc1(trange):
                    t0 = trange[0] * 128
                    tn = len(trange) * 128
                    with tc.tile_pool(name="wfc", bufs=6) as wfcp, \
                         tc.tile_pool(name="fcp", bufs=2,
                                      space="PSUM") as fcpp:
                        wfc_t = wfc_ext.ap().rearrange(
                            "p (m k j) -> p m (k j)", m=24, k=6)
                        for m in range(24):
                            wt = wfcp.tile([128, 6, 128], BF16, tag="wfc")
                            nc.sync.dma_start(out=wt, in_=wfc_t[:, m, :])
                            pf = fcpp.tile([128, 256], F32, tag="fc")
                            for k in range(6):
                                nc.tensor.matmul(
                                    pf, lhsT=wt[:, k, :],
                                    rhs=hT[:, k, t0:t0 + tn],
                                    start=(k == 0), stop=(k == 5))
                            nc.scalar.activation(
                                out=gT[:, m, t0:t0 + tn], in_=pf,
                                func=mybir.ActivationFunctionType
                                .Gelu_apprx_tanh,
                                bias=bfc_sb[:, m:m + 1])

                def mlp_fc2(trange):
                    with tc.tile_pool(name="f2p", bufs=1,
                                      space="PSUM") as f2pp, \
                         tc.tile_pool(name="osb", bufs=2) as osbp:
                        pf2s = {t: f2pp.tile([128, C], F32, tag=f"f2_{t}",
                                             name=f"pf2_{t}")
                                for t in trange}
                        for k in range(24):
                            for t in trange:
                                nc.tensor.matmul(
                                    pf2s[t][:, 0:512],
                                    lhsT=gT[:, k, t * 128:(t + 1) * 128],
                                    rhs=wfc2_sb[:, k, 0:512],
                                    start=(k == 0), stop=(k == 23))
                                nc.tensor.matmul(
                                    pf2s[t][:, 512:768],
                                    lhsT=gT[:, k, t * 128:(t + 1) * 128],
                                    rhs=wfc2_sb[:, k, 512:768],
                                    start=(k == 0), stop=(k == 23))
                        for t in trange:
                            o_sb = osbp.tile([128, C], F32, tag="osb",
                                             name="osb")
                            nc.vector.tensor_add(o_sb, x_sb[:, t, :],
                                                 pf2s[t])
                            if add_fc2_bias:
                                nc.vector.tensor_add(o_sb, o_sb,
                                                     bout_sb[:, 1, :])
                            nc.sync.dma_start(
                                out=out_ext[t * 128:(t + 1) * 128, :],
                                in_=o_sb)

                # shared PSUM pools for alpha/proj/beta (8-bank budget:
                # epp 4 + avp 2 + fc1's fcpp 2)
                with tc.tile_pool(name="ep", bufs=2, space="PSUM") as epp:
                  with tc.tile_pool(name="avp", bufs=2, space="PSUM") as avp:

                    # ---- alpha: kt<8 (everything q0 needs) ----
                    pavs = {}
                    pends = []

                    def emit_av_a(pend):
                        p, e_sb, g = pend
                        kt = g[0]
                        for h2 in range(2):
                            h = 2 * p + h2
                            nc.tensor.matmul(
                                pavs[h], lhsT=v_lhsT(kt, h),
                                rhs=e_sb[:, h2 * 512:h2 * 512 + 512],
                                start=(kt == 0), stop=(kt == 7),
                                skip_group_check=True)
                        if kt == 7:
                            for h2 in range(2):
                                h = 2 * p + h2
                                finalize(h, pavs[h][:, 0:256], 0, 256, epp)
                                nc.vector.tensor_copy(part1[:, h, :],
                                                      pavs[h][:, 256:512])
                                del pavs[h]

                    for p in range(6):
                        for h2 in range(2):
                            pavs[2 * p + h2] = avp.tile(
                                [65, TLOC], F32, tag="av",
                                name=f"pav{2 * p + h2}")
                        for g in GROUPS_A:
                            e_sb = do_group(p, g, avp, epp, None, None)
                            pends.append((p, e_sb, g))
                            if len(pends) > 3:
                                emit_av_a(pends.pop(0))
                    for pend in pends:
                        emit_av_a(pend)
                    pends = []

                    # ---- proj + residual + LN2 (psum from epp) ----
                    def proj_ln2(trange):
                        with tc.tile_pool(name="ln2", bufs=2) as ln2p:
                            for t in trange:
                                pp = epp.tile([128, C], F32, tag="e",
                                              name="pp")
                                for p in range(6):
                                    y_ap = yT[:, p, t * 128:(t + 1) * 128]
                                    nc.tensor.matmul(pp[:, 0:512], lhsT=y_ap,
                                                     rhs=wp_sb[:, p, 0:512],
                                                     start=(p == 0),
                                                     stop=(p == 5))
                                    nc.tensor.matmul(pp[:, 512:768],
                                                     lhsT=y_ap,
                                                     rhs=wp_sb[:, p,
                                                               512:768],
                                                     start=(p == 0),
                                                     stop=(p == 5))
                                nc.vector.tensor_add(x_sb[:, t, :],
                                                     x_sb[:, t, :], pp)
                                if add_proj_bias:
                                    nc.vector.tensor_add(x_sb[:, t, :],
                                                         x_sb[:, t, :],
                                                         bout_sb[:, 0, :])
                                xn2 = ln2p.tile([128, C], BF16, tag="xn2",
                                                name="xn2")
                                layernorm_to(ln2p, x_sb[:, t, :], xn2, "2")
                                for ct in range(6):
                                    pt = epp.tile([128, 128], BF16, tag="e",
                                                  name="pt")
                                    nc.tensor.transpose(
                                        pt,
                                        xn2[:, ct * 128:(ct + 1) * 128],
                                        ident)
                                    nc.vector.tensor_copy(
                                        hT[:, ct, t * 128:(t + 1) * 128],
                                        pt)

                    proj_ln2((0, 1))

                    # ---- beta: kt>=8 (q1's second half of keys) ----
                    pav2 = {}
                    pends = []

                    def emit_av_b(pend):
                        p, e_sb, g = pend
                        for h2 in range(2):
                            h = 2 * p + h2
                            for i, kt in enumerate(g):
                                so = h2 * 512 + i * 256
                                nc.tensor.matmul(
                                    pav2[h], lhsT=v_lhsT(kt, h),
                                    rhs=e_sb[:, so:so + 256],
                                    start=(kt == 8), stop=(kt == 15),
                                    skip_group_check=True)
                        if g[-1] == 15:
                            for h2 in range(2):
                                h = 2 * p + h2
                                sum_sb = small.tile([65, 256], F32,
                                                    tag="sum", name="sum")
                                nc.vector.tensor_add(sum_sb, pav2[h],
                                                     part1[:, h, :])
                                finalize(h, sum_sb, 256, 256, epp)
                                del pav2[h]

                    for p in range(6):
                        for h2 in range(2):
                            pav2[2 * p + h2] = avp.tile(
                                [65, 256], F32, tag="av",
                                name=f"pav2_{2 * p + h2}")
                        for g in GROUPS_B:
                            e_sb = do_group(p, g, avp, epp, None, None)
                            pends.append((p, e_sb, g))
                            if len(pends) > 3:
                                emit_av_b(pends.pop(0))
                    for pend in pends:
                        emit_av_b(pend)
                    pends = []

                    # ---- MLP fc1 for q0 (overlaps beta's exp wall) ----
                    mlp_fc1((0, 1))

                  # avp closed: fc2 psum (4 banks) now fits next to epp
                  mlp_fc2((0, 1))
                  proj_ln2((2, 3))
                  with tc.tile_pool(name="avp2", bufs=2,
                                    space="PSUM") as avp2:
                      mlp_fc1((2, 3))
                  mlp_fc2((2, 3))

    nc.compile()
    return nc


def _preprocess(inputs):
    f = lambda k: np.asarray(inputs[k], np.float32)
    x = f("x"); w_attn = f("w_attn"); b_attn = f("b_attn")
    w_proj = f("w_proj"); b_proj = f("b_proj")
    w_fc = f("w_fc"); b_fc = f("b_fc"); w_fc2 = f("w_fc2"); b_fc2 = f("b_fc2")
    ln1_g = f("ln1_g"); ln1_b = f("ln1_b"); ln2_g = f("ln2_g"); ln2_b = f("ln2_b")

    w_attn_eff = ln1_g[:, None] * w_attn
    b_attn_eff = b_attn + ln1_b @ w_attn
    s = 1.0 / np.sqrt(HD)
    w_q = w_attn_eff[:, 0:C] * s
    w_k = w_attn_eff[:, C:2 * C]
    w_v = w_attn_eff[:, 2 * C:3 * C]
    b_q = b_attn_eff[0:C] * s
    b_k = b_attn_eff[C:2 * C]
    b_v = b_attn_eff[2 * C:3 * C]
    b_proj_eff = b_proj + b_v @ w_proj
    w_fc_eff = ln2_g[:, None] * w_fc
    b_fc_eff = b_fc + ln2_b @ w_fc

    def pmajor(w, blocks):   # [C_in, N] -> [128, blocks * N/blocks...]
        # w: [(blocks*128), N] -> [128, blocks, N] p-major
        n = w.shape[1]
        return np.ascontiguousarray(
            w.reshape(blocks, 128, n).transpose(1, 0, 2).reshape(
                128, blocks * n).astype(BF))

    wq16 = pmajor(w_q, 6)
    wk16 = pmajor(w_k, 6)
    wv16 = pmajor(w_v, 6)
    wp16 = pmajor(w_proj, 6)
    # wfc: [128, 24 m, 6 k, 128] p-major
    wfc16 = np.ascontiguousarray(
        w_fc_eff.reshape(6, 128, 24, 128).transpose(1, 2, 0, 3).reshape(
            128, 24 * 6 * 128).astype(BF))
    wfc216 = pmajor(w_fc2, 24)

    bqk = np.stack([b_q, b_k]).astype(np.float32)
    bout = np.stack([b_proj_eff, b_fc2]).astype(np.float32)

    flags = (bool(np.any(bqk != 0)), bool(np.any(b_proj_eff != 0)),
             bool(np.any(b_fc2 != 0)))

    # mask slab [128, 6144] per core-group position j
    kpos = np.arange(128)
    qpos = np.arange(CHUNK)
    masks = np.zeros((4, 128, MASK_W), np.float32)
    for j in range(4):
        for kt in range(NKT):
            gk = kt * 128 + kpos[:, None]
            if kt < 8:
                off = kt * 512
                gq0 = j * CHUNK + qpos[None, :]
                gq1 = (7 - j) * CHUNK + qpos[None, :]
                masks[j, :, off:off + 256] = (gq0 >= gk)
                masks[j, :, off + 256:off + 512] = (gq1 >= gk)
            else:
                off = 4096 + (kt - 8) * 256
                gq1 = (7 - j) * CHUNK + qpos[None, :]
                masks[j, :, off:off + 256] = (gq1 >= gk)
    masks16 = masks.astype(BF)

    in_maps = []
    for c in range(NCORES):
        b, j = c // 4, c % 4
        x_loc = np.concatenate(
            [x[b, j * CHUNK:(j + 1) * CHUNK],
             x[b, (7 - j) * CHUNK:(8 - j) * CHUNK]]).astype(np.float32)
        x_pm = np.ascontiguousarray(
            x_loc.reshape(4, 128, C).transpose(1, 0, 2).reshape(128, 4 * C))
        in_maps.append({
            "x": x_pm,
            "wq": wq16, "wk": wk16, "wv": wv16, "wp": wp16,
            "wfc": wfc16, "wfc2": wfc216,
            "masks": np.ascontiguousarray(masks16[j]),
            "bqk": bqk, "bfc": b_fc_eff.astype(np.float32), "bout": bout,
        })
    return in_maps, flags


def kernel(**inputs):
    global LAST_EXEC_NS, LAST_RESULTS
    in_maps, flags = _preprocess(inputs)
    if flags not in _CACHE:
        _CACHE[flags] = _build(*flags)
    nc = _CACHE[flags]
    trace = bool(os.environ.get("BASS_KERNEL_TRACE"))
    res = run_bass_kernel_spmd(nc, in_maps, core_ids=list(range(NCORES)),
                               trace=trace)
    LAST_EXEC_NS = res.exec_time_ns
    LAST_RESULTS = res
    out = np.empty((B, T, C), np.float32)
    for c in range(NCORES):
        b, j = c // 4, c % 4
        o = res.results[c]["out"]
        out[b, j * CHUNK:(j + 1) * CHUNK] = o[0:CHUNK]
        out[b, (7 - j) * CHUNK:(8 - j) * CHUNK] = o[CHUNK:TLOC]
    return out


# revision 11
# speedup vs baseline: 1.1479x; 1.0598x over previous
"""GPT-2 style transformer block on 8 TRN2 NeuronCores.

Sharding: token-data-parallel. Each batch's 2048 tokens are split into 8
chunks of 256; core c owns batch c//4 and chunks {j, 7-j} (j = c%4) so
causal attention work is balanced. QKV/proj/MLP/LN are purely local; the
only collectives are four small AllGathers (k chunk0, v chunk0, k chunk1,
v chunk1) within each 4-core batch group, fp8 on the wire. Chunk-major
wire layout means kt<8 (everything q0 needs, and all of alpha below) is
available after the first two AGs. All bounce/unpack DMAs are p-major
contiguous (128 descriptors each).

Score matmuls are head-PAIR packed: heads 2p/2p+1 run as two concurrent
row-tiled K=64 matmuls (tile_position row groups 0/64, fp8 lhsT x bf16
rhs) writing the two banks of one [128,1024] PSUM tile; one exp covers
both heads, and the causal 0/1 mask is applied with a stride-0-broadcast
tensor_tensor so a single 512-wide mask slab serves both heads. proj is
pair-packed (K=128 = two heads' hd rows).

Attention is split alpha (kt<8) / beta (kt>=8): after alpha, q0's
(tokens 0:256) attention is final, so proj+LN2+MLP for q0 run while
beta's exp wall occupies the scalar engine. The v rides the wire with a
65th ones-column so softmax denominators come free in the av matmul;
softmax runs without max-subtraction; 1/sqrt(hd) is folded into w_q and
the v-bias into the proj bias.
"""

import os
import sys

sys.path.insert(0, "/opt/trn_rl_repo")

import numpy as np
import ml_dtypes

import concourse.bass as bass
import concourse.tile as tile
from concourse import bacc, mybir
from concourse.bass_utils import run_bass_kernel_spmd
from concourse.masks import make_identity

F32 = mybir.dt.float32
FP8 = mybir.dt.float8e4
BF16 = mybir.dt.bfloat16
BF = ml_dtypes.bfloat16

B, T, C, H, HD = 2, 2048, 768, 12, 64
EPS = 1e-5
NCORES = 8
CHUNK = 256            # global chunk size (tokens)
TLOC = 512             # local tokens per core (2 chunks)
NKT = T // 128         # 16 key tiles per batch
MASK_W = 8 * 512 + 8 * 256   # 6144

KH = 6 * 128 * 256     # k chunk: [128 p][6 ct][256 t]
VW = 780               # v wire row: 12 heads x 65 (64 + ones)
VH = 2 * 128 * VW      # v chunk: [128 p][2 tt][780]

GROUPS_A = [(kt,) for kt in range(8)]
GROUPS_B = [(8, 9), (10, 11), (12, 13), (14, 15)]

LAST_EXEC_NS = None
LAST_RESULTS = None
_CACHE = {}


def _build(add_qk_bias, add_proj_bias, add_fc2_bias):
    nc = bacc.Bacc("TRN2", target_bir_lowering=False, debug=False,
                   num_devices=NCORES)

    x_ext = nc.dram_tensor("x", [128, 4 * C], F32, kind="ExternalInput")
    wq_ext = nc.dram_tensor("wq", [128, 6 * C], BF16, kind="ExternalInput")
    wk_ext = nc.dram_tensor("wk", [128, 6 * C], BF16, kind="ExternalInput")
    wv_ext = nc.dram_tensor("wv", [128, 6 * C], BF16, kind="ExternalInput")
    wp_ext = nc.dram_tensor("wp", [128, 6 * C], BF16, kind="ExternalInput")
    wfc_ext = nc.dram_tensor("wfc", [128, 24 * 6 * 128], BF16,
                             kind="ExternalInput")
    wfc2_ext = nc.dram_tensor("wfc2", [128, 24 * C], BF16,
                              kind="ExternalInput")
    masks_ext = nc.dram_tensor("masks", [128, MASK_W], BF16,
                               kind="ExternalInput")
    bqk_ext = nc.dram_tensor("bqk", [2, C], F32, kind="ExternalInput")
    bfc_ext = nc.dram_tensor("bfc", [4 * C], F32, kind="ExternalInput")
    bout_ext = nc.dram_tensor("bout", [2, C], F32, kind="ExternalInput")
    out_ext = nc.dram_tensor("out", [TLOC, C], F32, kind="ExternalOutput")

    with tile.TileContext(nc) as tc:
        with tc.tile_pool(name="dram", bufs=1, space="DRAM") as dram, \
             tc.tile_pool(name="singles", bufs=1) as singles, \
             tc.tile_pool(name="persist", bufs=1) as persist, \
             tc.tile_pool(name="small", bufs=3) as small:

            kc_in = [dram.tile([KH], FP8, name=f"kc_in{i}")
                     for i in range(2)]
            kc_all = [dram.tile([4, KH], FP8, name=f"kc_all{i}")
                      for i in range(2)]
            vc_in = [dram.tile([VH], FP8, name=f"vc_in{i}")
                     for i in range(2)]
            vc_all = [dram.tile([4, VH], FP8, name=f"vc_all{i}")
                      for i in range(2)]

            # x first: LN1 needs it before anything else
            x_sb = persist.tile([128, 4, C], F32)     # local x, becomes xmid
            for t in range(4):
                nc.sync.dma_start(out=x_sb[:, t, :],
                                  in_=x_ext[:, t * C:(t + 1) * C])

            ident = singles.tile([128, 128], BF16)
            make_identity(nc, ident)
            eps_sb = singles.tile([128, 1], F32)
            nc.vector.memset(eps_sb, EPS)
            ones_pad = singles.tile([128, 64], F32)
            nc.vector.memset(ones_pad, 0.0)
            nc.vector.memset(ones_pad[0:1, :], 1.0)
            d_sb = singles.tile([128, 2, TLOC], F32)
            nc.vector.memset(d_sb, 1.0)

            bqk_sb = singles.tile([128, 2, 6], F32)
            if add_qk_bias:
                nc.sync.dma_start(
                    out=bqk_sb,
                    in_=bqk_ext.ap().rearrange("b (m p) -> p b m", p=128))
            bout_sb = singles.tile([128, 2, C], F32)
            if add_proj_bias or add_fc2_bias:
                bc = bout_ext.ap()
                nc.sync.dma_start(
                    out=bout_sb,
                    in_=bass.AP(tensor=bc.tensor, offset=bc.offset,
                                ap=[[0, 128], bc.ap[0], bc.ap[1]]))

            masks_sb = persist.tile([128, MASK_W], BF16)
            hT = persist.tile([128, 6, TLOC], BF16)   # h^T, reused for h2^T
            qT = persist.tile([128, 6, TLOC], BF16)   # head-pair layout
            yT = persist.tile([128, 6, TLOC], BF16)   # head-pair layout
            wp_sb = persist.tile([128, 6, C], BF16)
            wfc2_sb = persist.tile([128, 24, C], BF16)
            bfc_sb = singles.tile([128, 24], F32)
            gT = persist.tile([128, 24, TLOC], BF16)

            def layernorm_to(pool, xt, dst, tagsuf):
                stats = pool.tile([128, 3, 6], F32, tag="st" + tagsuf,
                                  name="st" + tagsuf)
                for sg in range(3):
                    nc.vector.bn_stats(out=stats[:, sg, :],
                                       in_=xt[:, sg * 256:(sg + 1) * 256])
                mv = pool.tile([128, 2], F32, tag="mv" + tagsuf,
                               name="mv" + tagsuf)
                nc.vector.bn_aggr(out=mv, in_=stats)
                nc.scalar.activation(out=mv[:, 1:2], in_=mv[:, 1:2],
                                     func=mybir.ActivationFunctionType.Sqrt,
                                     bias=eps_sb)
                nc.vector.reciprocal(out=mv[:, 1:2], in_=mv[:, 1:2])
                nc.vector.tensor_scalar(out=dst, in0=xt,
                                        scalar1=mv[:, 0:1], scalar2=mv[:, 1:2],
                                        op0=mybir.AluOpType.subtract,
                                        op1=mybir.AluOpType.mult)

            # ---------------- LN1 + transpose + QKV + AGs ----------------
            with tc.tile_pool(name="ln", bufs=3) as lnp, \
                 tc.tile_pool(name="tp", bufs=2, space="PSUM") as tpp, \
                 tc.tile_pool(name="qkp", bufs=2, space="PSUM") as qkp, \
                 tc.tile_pool(name="vp", bufs=2, space="PSUM") as vpp, \
                 tc.tile_pool(name="vsb", bufs=1) as vsbp:

                kT = vsbp.tile([128, 2, 6, 256], FP8)   # chunk-major
                v_sb = vsbp.tile([128, 4, VW], FP8)     # tt-major, with ones
                v5 = v_sb[:].rearrange("p t (h e) -> p t h e", e=65)
                nc.vector.memset(v5[:, :, :, 64:65], 1.0)
                wk_sb = vsbp.tile([128, 6, C], BF16)
                wq_sb = vsbp.tile([128, 6, C], BF16)
                wv_sb = vsbp.tile([128, 6, C], BF16)
                for sb, ext in ((wk_sb, wk_ext), (wv_sb, wv_ext),
                                (wq_sb, wq_ext)):
                    nc.sync.dma_start(
                        out=sb, in_=ext.ap().rearrange("p (a c) -> p a c",
                                                       c=C))

                def ln1_tile(t):
                    xn = lnp.tile([128, C], BF16, tag="xn")
                    layernorm_to(lnp, x_sb[:, t, :], xn, "1")
                    for ct in range(6):
                        pt = tpp.tile([128, 128], BF16, tag="tp")
                        nc.tensor.transpose(
                            pt, xn[:, ct * 128:(ct + 1) * 128], ident)
                        nc.scalar.copy(hT[:, ct, t * 128:(t + 1) * 128], pt)

                def k_chunk(cch):
                    sl = slice(cch * 256, cch * 256 + 256)
                    for m in range(6):
                        ps = qkp.tile([128, 256], F32, tag="qk", name="psk")
                        for k in range(6):
                            nc.tensor.matmul(
                                ps, lhsT=wk_sb[:, k, m * 128:(m + 1) * 128],
                                rhs=hT[:, k, sl], start=(k == 0),
                                stop=(k == 5))
                        if add_qk_bias:
                            nc.scalar.activation(
                                out=kT[:, cch, m, :], in_=ps,
                                func=mybir.ActivationFunctionType.Copy,
                                bias=bqk_sb[:, 1, m:m + 1])
                        else:
                            nc.scalar.copy(kT[:, cch, m, :], ps)
                    nc.sync.dma_start(
                        out=kc_in[cch][:].rearrange("(p ct t) -> p ct t",
                                                    p=128, t=256),
                        in_=kT[:, cch, :, :])
                    nc.gpsimd.collective_compute(
                        "AllGather", mybir.AluOpType.bypass,
                        replica_groups=[[0, 1, 2, 3], [4, 5, 6, 7]],
                        ins=[kc_in[cch][:].opt()],
                        outs=[kc_all[cch][:].opt()])

                def v_chunk(cch):
                    for tt in (2 * cch, 2 * cch + 1):
                        pv = vpp.tile([128, 1024], F32, tag="v")
                        for k in range(6):
                            nc.tensor.matmul(
                                pv[:, 0:384],
                                lhsT=hT[:, k, tt * 128:(tt + 1) * 128],
                                rhs=wv_sb[:, k, 0:384],
                                start=(k == 0), stop=(k == 5))
                            nc.tensor.matmul(
                                pv[:, 512:896],
                                lhsT=hT[:, k, tt * 128:(tt + 1) * 128],
                                rhs=wv_sb[:, k, 384:768],
                                start=(k == 0), stop=(k == 5))
                        nc.scalar.copy(
                            v5[:, tt, 0:6, 0:64],
                            pv[:, 0:384].rearrange("p (h e) -> p h e", e=64))
                        nc.scalar.copy(
                            v5[:, tt, 6:12, 0:64],
                            pv[:, 512:896].rearrange("p (h e) -> p h e",
                                                     e=64))
                    nc.sync.dma_start(
                        out=vc_in[cch][:].rearrange(
                            "(p t d) -> p t d", p=128, d=VW),
                        in_=v_sb[:, 2 * cch:2 * cch + 2, :])
                    nc.gpsimd.collective_compute(
                        "AllGather", mybir.AluOpType.bypass,
                        replica_groups=[[0, 1, 2, 3], [4, 5, 6, 7]],
                        ins=[vc_in[cch][:].opt()],
                        outs=[vc_all[cch][:].opt()])

                ln1_tile(0)
                ln1_tile(1)
                k_chunk(0)
                v_chunk(0)
                ln1_tile(2)
                ln1_tile(3)
                k_chunk(1)
                v_chunk(1)

                # q (pair layout: m-tile = head pair)
                for m in range(6):
                    ps = qkp.tile([128, TLOC], F32, tag="qk")
                    for k in range(6):
                        nc.tensor.matmul(
                            ps, lhsT=wq_sb[:, k, m * 128:(m + 1) * 128],
                            rhs=hT[:, k, :], start=(k == 0), stop=(k == 5))
                    if add_qk_bias:
                        nc.scalar.activation(
                            out=qT[:, m, :], in_=ps,
                            func=mybir.ActivationFunctionType.Copy,
                            bias=bqk_sb[:, 0, m:m + 1])
                    else:
                        nc.scalar.copy(qT[:, m, :], ps)

                # deferred weight/mask DMAs (needed later than x/w{k,v,q})
                nc.sync.dma_start(out=masks_sb, in_=masks_ext.ap())
                nc.sync.dma_start(
                    out=wp_sb,
                    in_=wp_ext.ap().rearrange("p (a c) -> p a c", c=C))
                nc.sync.dma_start(
                    out=wfc2_sb,
                    in_=wfc2_ext.ap().rearrange("p (a c) -> p a c", c=C))
                nc.sync.dma_start(
                    out=bfc_sb,
                    in_=bfc_ext.ap().rearrange("(m p) -> p m", p=128))

            # ---------------- attention ----------------
            with tc.tile_pool(name="kch", bufs=1) as kchp, \
                 tc.tile_pool(name="vaug", bufs=1) as vaugp, \
                 tc.tile_pool(name="esb", bufs=4) as esbp, \
                 tc.tile_pool(name="p1", bufs=1) as p1p:

                # k_ch: [p][chunk 2][rank 4][ct 6][256]
                k_ch = kchp.tile([128, 2, 4, 6, 256], FP8)
                # va4: [p][chunk 2][rank 4][tt 2][12*65] as wired
                v_ch = vaugp.tile([128, 2, 4, 2, VW], FP8)
                part1 = p1p.tile([65, 12, 256], F32)

                for cch in range(2):
                    for r in range(4):
                        nc.sync.dma_start(
                            out=k_ch[:, cch, r, :, :],
                            in_=kc_all[cch][r].rearrange(
                                "(p ct t) -> p ct t", p=128, t=256))
                        nc.gpsimd.dma_start(
                            out=v_ch[:, cch, r, :, :],
                            in_=vc_all[cch][r].rearrange(
                                "(p t d) -> p t d", p=128, d=VW))

                def k_lhsT(kt, p, half):
                    cch = 0 if kt < 8 else 1
                    ck = kt // 2
                    r = ck if ck < 4 else 7 - ck
                    loc = (kt % 2) * 128
                    return k_ch[64 * half:64 * half + 64, cch, r, p,
                                loc:loc + 128]

                def v_lhsT(kt, h):
                    cch = 0 if kt < 8 else 1
                    ck = kt // 2
                    r = ck if ck < 4 else 7 - ck
                    t = kt % 2
                    return v_ch[:, cch, r, t, h * 65:(h + 1) * 65]

                def finalize(h, src, q0, qn, pool):
                    h2, p = h % 2, h // 2
                    nc.vector.tensor_copy(d_sb[0:1, h2, q0:q0 + qn],
                                          src[64:65, :])
                    pb = pool.tile([64, qn], F32, tag="e", name="pbc")
                    nc.tensor.matmul(pb, lhsT=ones_pad,
                                     rhs=d_sb[:, h2, q0:q0 + qn],
                                     start=True, stop=True)
                    b_sb = small.tile([64, 256], F32, tag="bsb", name="bsb")
                    nc.vector.reciprocal_approx_fast(out=b_sb[:, 0:qn],
                                                     in_=pb)
                    nc.vector.tensor_mul(yT[64 * h2:64 * h2 + 64, p,
                                            q0:q0 + qn],
                                         src[0:64, :], b_sb[:, 0:qn])

                def do_group(p, g, pool, epool, q_sl, pav_w):
                    pe = epool.tile([128, 1024], F32, tag="e")
                    if len(g) == 1:
                        kt = g[0]
                        for half in range(2):
                            nc.tensor.matmul(
                                pe[:, half * 512:half * 512 + 512],
                                lhsT=k_lhsT(kt, p, half),
                                rhs=qT[64 * half:64 * half + 64, p, :],
                                start=True, stop=True)
                        moff = kt * 512
                    else:
                        for half in range(2):
                            for i, kt in enumerate(g):
                                so = half * 512 + i * 256
                                nc.tensor.matmul(
                                    pe[:, so:so + 256],
                                    lhsT=k_lhsT(kt, p, half),
                                    rhs=qT[64 * half:64 * half + 64, p,
                                           256:512],
                                    start=True, stop=True)
                        moff = 4096 + (g[0] - 8) * 256
                    e_sb = esbp.tile([128, 1024], BF16, tag="esb")
                    nc.scalar.activation(
                        out=e_sb, in_=pe,
                        func=mybir.ActivationFunctionType.Exp)
                    ms = masks_sb[:, moff:moff + 512]
                    mb = bass.AP(tensor=ms.tensor, offset=ms.offset,
                                 ap=[ms.ap[0], [0, 2], ms.ap[1]])
                    e3 = e_sb[:].rearrange("p (a c) -> p a c", a=2)
                    nc.vector.tensor_mul(e3, e3, mb)
                    return e_sb

                def mlp_fc1(trange):
                    t0 = trange[0] * 128
                    tn = len(trange) * 128
                    with tc.tile_pool(name="wfc", bufs=6) as wfcp, \
                         tc.tile_pool(name="fcp", bufs=2,
                                      space="PSUM") as fcpp:
                        wfc_t = wfc_ext.ap().rearrange(
                            "p (m k j) -> p m (k j)", m=24, k=6)
                        for m in range(24):
                            wt = wfcp.tile([128, 6, 128], BF16, tag="wfc")
                            nc.sync.dma_start(out=wt, in_=wfc_t[:, m, :])
                            pf = fcpp.tile([128, 256], F32, tag="fc")
                            for k in range(6):
                                nc.tensor.matmul(
                                    pf, lhsT=wt[:, k, :],
                                    rhs=hT[:, k, t0:t0 + tn],
                                    start=(k == 0), stop=(k == 5))
                            nc.scalar.activation(
                                out=gT[:, m, t0:t0 + tn], in_=pf,
                                func=mybir.ActivationFunctionType
                                .Gelu_apprx_tanh,
                                bias=bfc_sb[:, m:m + 1])

                def mlp_fc2(trange):
                    with tc.tile_pool(name="f2p", bufs=1,
                                      space="PSUM") as f2pp, \
                         tc.tile_pool(name="osb", bufs=2) as osbp:
                        pf2s = {t: f2pp.tile([128, C], F32, tag=f"f2_{t}",
                                             name=f"pf2_{t}")
                                for t in trange}
                        for k in range(24):
                            for t in trange:
                                nc.tensor.matmul(
                                    pf2s[t][:, 0:512],
                                    lhsT=gT[:, k, t * 128:(t + 1) * 128],
                                    rhs=wfc2_sb[:, k, 0:512],
                                    start=(k == 0), stop=(k == 23))
                                nc.tensor.matmul(
                                    pf2s[t][:, 512:768],
                                    lhsT=gT[:, k, t * 128:(t + 1) * 128],
                                    rhs=wfc2_sb[:, k, 512:768],
                                    start=(k == 0), stop=(k == 23))
                        for t in trange:
                            o_sb = osbp.tile([128, C], F32, tag="osb",
                                             name="osb")
                            nc.vector.tensor_add(o_sb, x_sb[:, t, :],
                                                 pf2s[t])
                            if add_fc2_bias:
                                nc.vector.tensor_add(o_sb, o_sb,
                                                     bout_sb[:, 1, :])
                            nc.sync.dma_start(
                                out=out_ext[t * 128:(t + 1) * 128, :],
                                in_=o_sb)

                # shared PSUM pools for alpha/proj/beta (8-bank budget:
                # epp 4 + avp 2 + fc1's fcpp 2)
                with tc.tile_pool(name="ep", bufs=2, space="PSUM") as epp:
                  with tc.tile_pool(name="avp", bufs=2, space="PSUM") as avp:

                    # ---- alpha: kt<8 (everything q0 needs) ----
                    pavs = {}
                    pends = []

                    def emit_av_a(pend):
                        p, e_sb, g = pend
                        kt = g[0]
                        for h2 in range(2):
                            h = 2 * p + h2
                            nc.tensor.matmul(
                                pavs[h], lhsT=v_lhsT(kt, h),
                                rhs=e_sb[:, h2 * 512:h2 * 512 + 512],
                                start=(kt == 0), stop=(kt == 7),
                                skip_group_check=True)
                        if kt == 7:
                            for h2 in range(2):
                                h = 2 * p + h2
                                finalize(h, pavs[h][:, 0:256], 0, 256, epp)
                                nc.vector.tensor_copy(part1[:, h, :],
                                                      pavs[h][:, 256:512])
                                del pavs[h]

                    for p in range(6):
                        for h2 in range(2):
                            pavs[2 * p + h2] = avp.tile(
                                [65, TLOC], F32, tag="av",
                                name=f"pav{2 * p + h2}")
                        for g in GROUPS_A:
                            e_sb = do_group(p, g, avp, epp, None, None)
                            pends.append((p, e_sb, g))
                            if len(pends) > 3:
                                emit_av_a(pends.pop(0))
                    for pend in pends:
                        emit_av_a(pend)
                    pends = []

                    # ---- proj + residual + LN2 (psum from epp) ----
                    def proj_ln2(trange):
                        with tc.tile_pool(name="ln2", bufs=2) as ln2p:
                            for t in trange:
                                pp = epp.tile([128, C], F32, tag="e",
                                              name="pp")
                                for p in range(6):
                                    y_ap = yT[:, p, t * 128:(t + 1) * 128]
                                    nc.tensor.matmul(pp[:, 0:512], lhsT=y_ap,
                                                     rhs=wp_sb[:, p, 0:512],
                                                     start=(p == 0),
                                                     stop=(p == 5))
                                    nc.tensor.matmul(pp[:, 512:768],
                                                     lhsT=y_ap,
                                                     rhs=wp_sb[:, p,
                                                               512:768],
                                                     start=(p == 0),
                                                     stop=(p == 5))
                                nc.vector.tensor_add(x_sb[:, t, :],
                                                     x_sb[:, t, :], pp)
                                if add_proj_bias:
                                    nc.vector.tensor_add(x_sb[:, t, :],
                                                         x_sb[:, t, :],
                                                         bout_sb[:, 0, :])
                                xn2 = ln2p.tile([128, C], BF16, tag="xn2",
                                                name="xn2")
                                layernorm_to(ln2p, x_sb[:, t, :], xn2, "2")
                                for ct in range(6):
                                    pt = epp.tile([128, 128], BF16, tag="e",
                                                  name="pt")
                                    nc.tensor.transpose(
                                        pt,
                                        xn2[:, ct * 128:(ct + 1) * 128],
                                        ident)
                                    nc.vector.tensor_copy(
                                        hT[:, ct, t * 128:(t + 1) * 128],
                                        pt)

                    proj_ln2((0, 1))

                    # ---- beta: kt>=8 (q1's second half of keys) ----
                    pav2 = {}
                    pends = []

                    def emit_av_b(pend):
                        p, e_sb, g = pend
                        for h2 in range(2):
                            h = 2 * p + h2
                            for i, kt in enumerate(g):
                                so = h2 * 512 + i * 256
                                nc.tensor.matmul(
                                    pav2[h], lhsT=v_lhsT(kt, h),
                                    rhs=e_sb[:, so:so + 256],
                                    start=(kt == 8), stop=(kt == 15),
                                    skip_group_check=True)
                        if g[-1] == 15:
                            for h2 in range(2):
                                h = 2 * p + h2
                                sum_sb = small.tile([65, 256], F32,
                                                    tag="sum", name="sum")
                                nc.vector.tensor_add(sum_sb, pav2[h],
                                                     part1[:, h, :])
                                finalize(h, sum_sb, 256, 256, epp)
                                del pav2[h]

                    for p in range(6):
                        for h2 in range(2):
                            pav2[2 * p + h2] = avp.tile(
                                [65, 256], F32, tag="av",
                                name=f"pav2_{2 * p + h2}")
                        for g in GROUPS_B:
                            e_sb = do_group(p, g, avp, epp, None, None)
                            pends.append((p, e_sb, g))
                            if len(pends) > 3:
                                emit_av_b(pends.pop(0))
                    for pend in pends:
                        emit_av_b(pend)
                    pends = []

                    # ---- MLP fc1 for q0 (overlaps beta's exp wall) ----
                    mlp_fc1((0, 1))

                  # avp closed: fc2 psum (4 banks) now fits next to epp
                  proj_ln2((2, 3))
                  mlp_fc2((0, 1))
                  mlp_fc1((2, 3))
                  mlp_fc2((2, 3))

    nc.compile()
    return nc


def _preprocess(inputs):
    f = lambda k: np.asarray(inputs[k], np.float32)
    x = f("x"); w_attn = f("w_attn"); b_attn = f("b_attn")
    w_proj = f("w_proj"); b_proj = f("b_proj")
    w_fc = f("w_fc"); b_fc = f("b_fc"); w_fc2 = f("w_fc2"); b_fc2 = f("b_fc2")
    ln1_g = f("ln1_g"); ln1_b = f("ln1_b"); ln2_g = f("ln2_g"); ln2_b = f("ln2_b")

    w_attn_eff = ln1_g[:, None] * w_attn
    b_attn_eff = b_attn + ln1_b @ w_attn
    s = 1.0 / np.sqrt(HD)
    w_q = w_attn_eff[:, 0:C] * s
    w_k = w_attn_eff[:, C:2 * C]
    w_v = w_attn_eff[:, 2 * C:3 * C]
    b_q = b_attn_eff[0:C] * s
    b_k = b_attn_eff[C:2 * C]
    b_v = b_attn_eff[2 * C:3 * C]
    b_proj_eff = b_proj + b_v @ w_proj
    w_fc_eff = ln2_g[:, None] * w_fc
    b_fc_eff = b_fc + ln2_b @ w_fc

    def pmajor(w, blocks):   # [C_in, N] -> [128, blocks * N/blocks...]
        # w: [(blocks*128), N] -> [128, blocks, N] p-major
        n = w.shape[1]
        return np.ascontiguousarray(
            w.reshape(blocks, 128, n).transpose(1, 0, 2).reshape(
                128, blocks * n).astype(BF))

    wq16 = pmajor(w_q, 6)
    wk16 = pmajor(w_k, 6)
    wv16 = pmajor(w_v, 6)
    wp16 = pmajor(w_proj, 6)
    # wfc: [128, 24 m, 6 k, 128] p-major
    wfc16 = np.ascontiguousarray(
        w_fc_eff.reshape(6, 128, 24, 128).transpose(1, 2, 0, 3).reshape(
            128, 24 * 6 * 128).astype(BF))
    wfc216 = pmajor(w_fc2, 24)

    bqk = np.stack([b_q, b_k]).astype(np.float32)
    bout = np.stack([b_proj_eff, b_fc2]).astype(np.float32)

    flags = (bool(np.any(bqk != 0)), bool(np.any(b_proj_eff != 0)),
             bool(np.any(b_fc2 != 0)))

    # mask slab [128, 6144] per core-group position j
    kpos = np.arange(128)
    qpos = np.arange(CHUNK)
    masks = np.zeros((4, 128, MASK_W), np.float32)
    for j in range(4):
        for kt in range(NKT):
            gk = kt * 128 + kpos[:, None]
            if kt < 8:
                off = kt * 512
                gq0 = j * CHUNK + qpos[None, :]
                gq1 = (7 - j) * CHUNK + qpos[None, :]
                masks[j, :, off:off + 256] = (gq0 >= gk)
                masks[j, :, off + 256:off + 512] = (gq1 >= gk)
            else:
                off = 4096 + (kt - 8) * 256
                gq1 = (7 - j) * CHUNK + qpos[None, :]
                masks[j, :, off:off + 256] = (gq1 >= gk)
    masks16 = masks.astype(BF)

    in_maps = []
    for c in range(NCORES):
        b, j = c // 4, c % 4
        x_loc = np.concatenate(
            [x[b, j * CHUNK:(j + 1) * CHUNK],
             x[b, (7 - j) * CHUNK:(8 - j) * CHUNK]]).astype(np.float32)
        x_pm = np.ascontiguousarray(
            x_loc.reshape(4, 128, C).transpose(1, 0, 2).reshape(128, 4 * C))
        in_maps.append({
            "x": x_pm,
            "wq": wq16, "wk": wk16, "wv": wv16, "wp": wp16,
            "wfc": wfc16, "wfc2": wfc216,
            "masks": np.ascontiguousarray(masks16[j]),
            "bqk": bqk, "bfc": b_fc_eff.astype(np.float32), "bout": bout,
        })
    return in_maps, flags


def kernel(**inputs):
    global LAST_EXEC_NS, LAST_RESULTS
    in_maps, flags = _preprocess(inputs)
    if flags not in _CACHE:
        _CACHE[flags] = _build(*flags)
    nc = _CACHE[flags]
    trace = bool(os.environ.get("BASS_KERNEL_TRACE"))
    res = run_bass_kernel_spmd(nc, in_maps, core_ids=list(range(NCORES)),
                               trace=trace)
    LAST_EXEC_NS = res.exec_time_ns
    LAST_RESULTS = res
    out = np.empty((B, T, C), np.float32)
    for c in range(NCORES):
        b, j = c // 4, c % 4
        o = res.results[c]["out"]
        out[b, j * CHUNK:(j + 1) * CHUNK] = o[0:CHUNK]
        out[b, (7 - j) * CHUNK:(8 - j) * CHUNK] = o[CHUNK:TLOC]
    return out
